# revision 13
# baseline (speedup 1.0000x reference)
"""Trainium2 Bass kernel for nn_AdaptiveHyperNN (gnn_message_passing).

Math: the reference builds fully-connected edge features [B,N,N,D] through
linear layers and mean-aggregates.  Every edge MLP is linear before
aggregation, so the computation collapses to per-node work:

  feat   = api_embeds[invoked]                       (indirect-DMA gather)
  e2n1_v = fbar @ W1a + feat_v @ W1b + b1            (fbar = mean_v feat_v)
  h_v    = feat_v @ W2a + e2n1_v @ W2b + b2
  logit[u*N+v] = p[u] + q[v] + c,  p = h @ (W3a@W4a), q = h @ (W3b@W4a),
                 c = b3 @ W4a + Xs @ W4b + b4
  out = sigmoid(q broadcast + p as per-partition ACT bias)

Sharding: data-parallel over B (8 graphs -> 8 cores), weights replicated
(bf16-packed, two HWDGE rings).  Raw bacc with hand-placed semaphores.

Same math as kernel.py, but with hand-placed semaphores so that:
- no Tile prologue (sem-reset memsets / ordering modes / drains)
- the indirect gather completion is tracked by a DMA semaphore instead of
  a full gpsimd drain
- the sigmoid + output DMA are split in half so the second half's
  compute overlaps the first half's DMA receipt
- no Tile kernel-tail drain + EVSEM butterfly

Engine plan (sem = semaphore, +16 per DMA, +1 per compute op):
  sync   : DMA inv(dS16) wa(dS32) wc(dS48); [sA1] out0(dS64); [sA2] out1(dS80)
  scalar : DMA sm(dA16) wb(dA32); sigmoid warmup; [sP16,sV20] sig0(sA1) sig1(sA2)
  gpsimd : ident memset+affine(sG1); [dS16] indirect gather feat (dG16)
  tensor : [sG1,dG16] tr0(1) tr1(2) [dS32,sV4] e0(3) e1(4) [dS48,sV2] pw0(5)
           pw1(6) [sV9] pw2(7) [sV10] pw3(8) pc(9) [sV11] t10(10) [sV12] t11(11)
           [sV17,dA32] h0(12) h1(13) [sV19] p(14) q(15) [sV21] outer(16)
  vector : ones(1) [dA16] w4bf(2) [sP1] fT0(3) [sP2] fT1(4) fb0(5) fb1(6)
           fbb0(7) fbb1(8) [sP5] w34cc(9) [sP6] (10) [sP7] (11) [sP8] (12)
           [sP9] c(13) [sP10] eb0(14) [sP11] eb1(15) [sP3] e2T0(16) [sP4]
           e2T1(17) [sP12] hT0(18) [sP13] hT1(19) [sP14] p_sb(20) [sP15] q_row(21)
"""

import numpy as np
import ml_dtypes

import concourse.bacc as bacc
import concourse.bass as bass
import concourse.mybir as mybir

P = 128
D = 256
N = 128
B = 8
V = 10000
F32 = mybir.dt.float32
BF16 = mybir.dt.bfloat16
I32 = mybir.dt.int32

_W1 = 0
_W2 = 1024
_W3T = 2048
PKW = 3328  # 3072 weights + 256 b2-row (partition 0)
PKS = 13


def build_nc():
    nc = bacc.Bacc("TRN2", target_bir_lowering=False)
    AO = mybir.AluOpType

    inv = nc.dram_tensor("invoked", [N, 1], I32, kind="ExternalInput")
    emb = nc.dram_tensor("emb", [V, D], F32, kind="ExternalInput")
    pkw = nc.dram_tensor("pkw", [P, PKW], BF16, kind="ExternalInput")
    pks = nc.dram_tensor("pks", [P, PKS], F32, kind="ExternalInput")
    out = nc.dram_tensor("out", [N * N, 1], F32, kind="ExternalOutput")

    sb = nc.alloc_sbuf_tensor
    ident = sb("ident", [P, P], F32)
    inv_t = sb("inv_t", [P, 1], I32)
    feat = sb("feat", [P, D], F32)
    sm_sb = sb("sm_sb", [P, PKS], F32)
    wa_sb = sb("wa_sb", [P, 1024], BF16)
    wb_sb = sb("wb_sb", [P, 1024], BF16)
    wc_sb = sb("wc_sb", [P, 1280], BF16)
    w4bf = sb("w4bf", [P, 2], BF16)
    ones_row = sb("ones_row", [1, P], BF16)
    ones_col = sb("ones_col", [P, 1], F32)
    featT = [sb(f"featT{i}", [P, P], BF16) for i in range(2)]
    fbar32 = [sb(f"fbar32_{i}", [P, 1], F32) for i in range(2)]
    fbar = [sb(f"fbar{i}", [P, 1], BF16) for i in range(2)]
    e_bias = [sb(f"ebias{i}", [P, 1], F32) for i in range(2)]
    e2n1T = [sb(f"e2n1T{i}", [P, P], BF16) for i in range(2)]
    hT = [sb(f"hT{i}", [P, P], BF16) for i in range(2)]
    w34c = [sb(f"w34c{i}", [P, 2], BF16) for i in range(2)]
    c_sb = sb("c_sb", [1, 1], F32)
    p_sb = sb("p_sb", [P, 1], F32)
    q_row = sb("q_row", [1, P], BF16)
    osb = sb("osb", [P, P], F32)
    warm2 = sb("warm2", [1, 1], F32)

    pp = nc.alloc_psum_tensor
    PB0 = pp("PB0", [P, P], F32)
    PB1 = pp("PB1", [P, P], F32)
    PB2 = pp("PB2", [P, P], F32)
    PB3 = pp("PB3", [P, P], F32)
    PS0 = pp("PS0", [P, 1], F32)
    PS1 = pp("PS1", [P, 1], F32)
    PCP = pp("PCP", [P, 1], F32)
    PQ = pp("PQ", [1, P], F32)

    def w1s(kt, mt):
        return wa_sb[:, kt * 256 + mt * 128 : kt * 256 + (mt + 1) * 128]

    def w2s(kt, mt):
        return wb_sb[:, kt * 256 + mt * 128 : kt * 256 + (mt + 1) * 128]

    def w3s(jt, it):
        return wc_sb[:, jt * 512 + it * 128 : jt * 512 + (it + 1) * 128]

    def w4s(kt):
        return sm_sb[:, kt : kt + 1]

    def b1s(k):
        return sm_sb[:, 4 + k : 5 + k]

    def b2s(k):
        return sm_sb[:, 6 + k : 7 + k]

    def b3s(k):
        return sm_sb[:, 8 + k : 9 + k]

    def xss(k):
        return sm_sb[:, 10 + k : 11 + k]

    TSF = mybir.ActivationFunctionType

    with (
        nc.Block() as block,
        nc.semaphore("dI") as dI,
        nc.semaphore("dWA") as dWA,
        nc.semaphore("dWC") as dWC,
        nc.semaphore("dSM") as dSM,
        nc.semaphore("dWB") as dWB,
        nc.semaphore("dOUT") as dOUT,
        nc.semaphore("dG") as dG,
        nc.semaphore("sG") as sG,
        nc.semaphore("sP") as sP,
        nc.semaphore("sV") as sV,
        nc.semaphore("sA") as sA,
    ):

        @block.sync
        def _(sync):
            sync.dma_start(out=inv_t[:], in_=inv[:, :], single_packet=True).then_inc(dI, 16)
            sync.dma_start(out=wc_sb[:], in_=pkw[:, _W3T : _W3T + 1280]).then_inc(dWC, 16)
            sync.dma_start(out=wa_sb[:], in_=pkw[:, _W1 : _W1 + 1024]).then_inc(dWA, 16)
            sync.wait_ge(sA, 1)
            sync.dma_start(
                out=out[0 : 64 * N, :].rearrange("(u v) o -> u (v o)", v=N),
                in_=osb[0:64, :],
            ).then_inc(dOUT, 16)
            sync.wait_ge(sA, 2)
            sync.dma_start(
                out=out[64 * N : 128 * N, :].rearrange("(u v) o -> u (v o)", v=N),
                in_=osb[64:128, :],
            ).then_inc(dOUT, 16)
            sync.wait_ge(dOUT, 32)

        @block.scalar
        def _(scalar):
            scalar.dma_start(out=sm_sb[:], in_=pks[:, :]).then_inc(dSM, 16)
            scalar.dma_start(out=wb_sb[:], in_=pkw[:, _W2 : _W2 + 1024]).then_inc(dWB, 16)
            scalar.wait_ge(dSM, 16)
            nc.scalar.activation(out=warm2[:], in_=sm_sb[0:1, 0:1], func=TSF.Sigmoid, bias=sm_sb[0:1, 1:2])
            scalar.wait_ge(sP, 18)
            scalar.wait_ge(sV, 18)
            nc.scalar.activation(
                out=osb[0:64, :], in_=PB2[0:64, :], func=TSF.Sigmoid,
                bias=p_sb[0:64, :1],
            ).then_inc(sA, 1)
            scalar.wait_ge(sP, 19)
            nc.scalar.activation(
                out=osb[64:128, :], in_=PB2[64:128, :], func=TSF.Sigmoid,
                bias=p_sb[64:128, :1],
            ).then_inc(sA, 1)

        @block.gpsimd
        def _(gpsimd):
            gpsimd.memset(ident[:], 0.0)
            gpsimd.drain()
            gpsimd.affine_select(
                out=ident[:],
                in_=ident[:],
                compare_op=mybir.AluOpType.not_equal,
                fill=1.0,
                base=0,
                pattern=[[-1, P]],
                channel_multiplier=1,
            ).then_inc(sG, 1)
            gpsimd.wait_ge(dI, 16)
            gpsimd.indirect_dma_start(
                out=feat[:],
                out_offset=None,
                in_=emb[:, :],
                in_offset=bass.IndirectOffsetOnAxis(ap=inv_t[:, :1], axis=0),
            ).then_inc(dG, 16)

        @block.tensor
        def _(tensor):
            mm = nc.tensor.matmul
            # w34 + c first: they only need weights, run during the gather wait
            tensor.wait_ge(dWC, 16)
            tensor.wait_ge(sV, 2)
            for it, ps_ in ((0, PS0), (1, PS1)):
                mm(out=ps_[:], lhsT=w3s(0, it), rhs=w4bf[:, 0:1], start=True, stop=False)
                mm(out=ps_[:], lhsT=w3s(1, it), rhs=w4bf[:, 1:2], start=False, stop=True).then_inc(sP, 1)
            tensor.wait_ge(sV, 3)
            mm(out=PS0[:], lhsT=w3s(0, 2), rhs=w4bf[:, 0:1], start=True, stop=False)
            mm(out=PS0[:], lhsT=w3s(1, 2), rhs=w4bf[:, 1:2], start=False, stop=True).then_inc(sP, 1)
            tensor.wait_ge(sV, 4)
            mm(out=PS1[:], lhsT=w3s(0, 3), rhs=w4bf[:, 0:1], start=True, stop=False)
            mm(out=PS1[:], lhsT=w3s(1, 3), rhs=w4bf[:, 1:2], start=False, stop=True).then_inc(sP, 1)
            mm(out=PCP[0:1, :], lhsT=b3s(0), rhs=w4s(0), start=True, stop=False)
            mm(out=PCP[0:1, :], lhsT=b3s(1), rhs=w4s(1), start=False, stop=False)
            mm(out=PCP[0:1, :], lhsT=xss(0), rhs=w4s(2), start=False, stop=False)
            mm(out=PCP[0:1, :], lhsT=xss(1), rhs=w4s(3), start=False, stop=True).then_inc(sP, 1)
            # gather-dependent chain; trXb computes fbar sums via ones column
            tensor.wait_ge(sG, 1)
            tensor.wait_ge(dG, 16)
            nc.tensor.transpose(out=PB0[:], in_=feat[:, 0:P], identity=ident[:]).then_inc(sP, 1)
            nc.tensor.transpose(out=PB1[:], in_=feat[:, P : 2 * P], identity=ident[:]).then_inc(sP, 1)
            tensor.wait_ge(sV, 5)
            mm(out=PS0[:], lhsT=feat[:, 0:P], rhs=ones_col[:], start=True, stop=True).then_inc(sP, 1)
            tensor.wait_ge(sV, 6)
            mm(out=PS1[:], lhsT=feat[:, P : 2 * P], rhs=ones_col[:], start=True, stop=True).then_inc(sP, 1)
            tensor.wait_ge(dWA, 16)
            tensor.wait_ge(sV, 8)
            mm(out=PB2[:], lhsT=w1s(2, 0), rhs=featT[0][:], start=True, stop=False)
            mm(out=PB3[:], lhsT=w1s(2, 1), rhs=featT[0][:], start=True, stop=False)
            tensor.wait_ge(sV, 9)
            mm(out=PB2[:], lhsT=w1s(3, 0), rhs=featT[1][:], start=False, stop=True).then_inc(sP, 1)
            mm(out=PB3[:], lhsT=w1s(3, 1), rhs=featT[1][:], start=False, stop=True).then_inc(sP, 1)
            tensor.wait_ge(sV, 11)
            mm(out=PS0[:], lhsT=w1s(0, 0), rhs=fbar[0][:], start=True, stop=False)
            mm(out=PS0[:], lhsT=w1s(1, 0), rhs=fbar[1][:], start=False, stop=True).then_inc(sP, 1)
            mm(out=PS1[:], lhsT=w1s(0, 1), rhs=fbar[0][:], start=True, stop=False)
            mm(out=PS1[:], lhsT=w1s(1, 1), rhs=fbar[1][:], start=False, stop=True).then_inc(sP, 1)
            tensor.wait_ge(dWB, 16)
            mm(out=PB0[:], lhsT=wc_sb[0:1, 1024:1152], rhs=ones_row[:], start=True, stop=False)
            mm(out=PB1[:], lhsT=wc_sb[0:1, 1152:1280], rhs=ones_row[:], start=True, stop=False)
            mm(out=PB0[:], lhsT=w2s(0, 0), rhs=featT[0][:], start=False, stop=False)
            mm(out=PB0[:], lhsT=w2s(1, 0), rhs=featT[1][:], start=False, stop=False)
            mm(out=PB1[:], lhsT=w2s(0, 1), rhs=featT[0][:], start=False, stop=False)
            mm(out=PB1[:], lhsT=w2s(1, 1), rhs=featT[1][:], start=False, stop=False)
            tensor.wait_ge(sV, 14)
            mm(out=PB0[:], lhsT=w2s(2, 0), rhs=e2n1T[0][:], start=False, stop=False)
            mm(out=PB1[:], lhsT=w2s(2, 1), rhs=e2n1T[0][:], start=False, stop=False)
            tensor.wait_ge(sV, 15)
            mm(out=PB0[:], lhsT=w2s(3, 0), rhs=e2n1T[1][:], start=False, stop=True).then_inc(sP, 1)
            mm(out=PB1[:], lhsT=w2s(3, 1), rhs=e2n1T[1][:], start=False, stop=True).then_inc(sP, 1)
            tensor.wait_ge(sV, 16)
            mm(out=PCP[:], lhsT=hT[0][:], rhs=w34c[0][:, 0:1], start=True, stop=False)
            tensor.wait_ge(sV, 17)
            mm(out=PCP[:], lhsT=hT[1][:], rhs=w34c[1][:, 0:1], start=False, stop=True).then_inc(sP, 1)
            mm(out=PQ[:], lhsT=w34c[0][:, 1:2], rhs=hT[0][:], start=True, stop=False)
            mm(out=PQ[:], lhsT=w34c[1][:, 1:2], rhs=hT[1][:], start=False, stop=True).then_inc(sP, 1)
            tensor.wait_ge(sV, 19)
            mm(out=PB2[0:64, :], lhsT=ones_row[:, 0:64], rhs=q_row[:], start=True, stop=True).then_inc(sP, 1)
            mm(out=PB2[64:128, :], lhsT=ones_row[:, 64:128], rhs=q_row[:], start=True, stop=True).then_inc(sP, 1)

        @block.vector
        def _(vector):
            nc.vector.memset(ones_row[:], 1.0).then_inc(sV, 1)
            vector.wait_ge(dSM, 16)
            nc.vector.tensor_copy(out=w4bf[:], in_=sm_sb[:, 0:2]).then_inc(sV, 1)
            nc.vector.memset(ones_col[:], 1.0)
            vector.wait_ge(sP, 1)
            nc.vector.tensor_copy(out=w34c[0][:, 0:1], in_=PS0[:]).then_inc(sV, 1)
            vector.wait_ge(sP, 2)
            nc.vector.tensor_copy(out=w34c[1][:, 0:1], in_=PS1[:]).then_inc(sV, 1)
            vector.wait_ge(sP, 3)
            nc.vector.tensor_copy(out=w34c[0][:, 1:2], in_=PS0[:]).then_inc(sV, 1)
            vector.wait_ge(sP, 4)
            nc.vector.tensor_copy(out=w34c[1][:, 1:2], in_=PS1[:]).then_inc(sV, 1)
            vector.wait_ge(sP, 5)
            nc.vector.tensor_add(out=c_sb[:], in0=PCP[0:1, :], in1=sm_sb[0:1, 12:13]).then_inc(sV, 1)
            vector.wait_ge(sP, 6)
            nc.vector.tensor_copy(out=featT[0][:], in_=PB0[:]).then_inc(sV, 1)
            vector.wait_ge(sP, 7)
            nc.vector.tensor_copy(out=featT[1][:], in_=PB1[:]).then_inc(sV, 1)
            vector.wait_ge(sP, 8)
            nc.vector.tensor_copy(out=fbar[0][:], in_=PS0[:]).then_inc(sV, 1)
            vector.wait_ge(sP, 9)
            nc.vector.tensor_copy(out=fbar[1][:], in_=PS1[:]).then_inc(sV, 1)
            vector.wait_ge(sP, 12)
            nc.vector.tensor_scalar(
                out=e_bias[0][:], in0=PS0[:], scalar1=1.0 / N, scalar2=b1s(0),
                op0=AO.mult, op1=AO.add,
            ).then_inc(sV, 1)
            vector.wait_ge(sP, 13)
            nc.vector.tensor_scalar(
                out=e_bias[1][:], in0=PS1[:], scalar1=1.0 / N, scalar2=b1s(1),
                op0=AO.mult, op1=AO.add,
            ).then_inc(sV, 1)
            vector.wait_ge(sP, 10)
            vector.wait_ge(sV, 12)
            nc.vector.tensor_scalar_add(out=e2n1T[0][:], in0=PB2[:], scalar1=e_bias[0][:, :1]).then_inc(sV, 1)
            vector.wait_ge(sP, 11)
            vector.wait_ge(sV, 13)
            nc.vector.tensor_scalar_add(out=e2n1T[1][:], in0=PB3[:], scalar1=e_bias[1][:, :1]).then_inc(sV, 1)
            vector.wait_ge(sP, 14)
            nc.vector.tensor_copy(out=hT[0][:], in_=PB0[:]).then_inc(sV, 1)
            vector.wait_ge(sP, 15)
            nc.vector.tensor_copy(out=hT[1][:], in_=PB1[:]).then_inc(sV, 1)
            vector.wait_ge(sP, 16)
            nc.vector.tensor_copy(out=p_sb[:], in_=PCP[:]).then_inc(sV, 1)
            vector.wait_ge(sP, 17)
            vector.wait_ge(sV, 7)
            nc.vector.tensor_scalar_add(out=q_row[:], in0=PQ[:], scalar1=c_sb[:1, :1]).then_inc(sV, 1)

    import concourse.mybir as _mb
    for bb in nc.m.functions[0].blocks:
        if bb.name == "main":
            bb.instructions = [
                i for i in bb.instructions
                if not i.name.startswith("barrier_")
                and not isinstance(i, _mb.InstDrain)
            ]
        elif bb.name.endswith("_end"):
            bb.instructions = [
                i for i in bb.instructions if not i.name.startswith("barrier_")
            ]
    nc.compile()
    return nc


TRACE = False
LAST_RESULTS = None
_NC_CACHE = {}


def _pack_w(W1, W2, W3, b2):
    pkv = np.zeros((P, PKW), dtype=ml_dtypes.bfloat16)
    for kt in range(4):
        pkv[:, _W1 + kt * 256 : _W1 + (kt + 1) * 256] = W1[kt * P : (kt + 1) * P, :]
        pkv[:, _W2 + kt * 256 : _W2 + (kt + 1) * 256] = W2[kt * P : (kt + 1) * P, :]
    W3T = W3.T
    for jt in range(2):
        pkv[:, _W3T + jt * 512 : _W3T + (jt + 1) * 512] = W3T[jt * P : (jt + 1) * P, :]
    pkv[0, 3072:3328] = b2
    return pkv


def _pack_s(W4, b1, b2, b3, b4, Xs_b):
    pkv = np.zeros((P, PKS), dtype=np.float32)
    for kt in range(4):
        pkv[:, kt] = W4[kt * P : (kt + 1) * P, 0]
    for jt in range(2):
        pkv[:, 4 + jt] = b1[jt * P : (jt + 1) * P]
        pkv[:, 6 + jt] = b2[jt * P : (jt + 1) * P]
        pkv[:, 8 + jt] = b3[jt * P : (jt + 1) * P]
        pkv[:, 10 + jt] = Xs_b[jt * P : (jt + 1) * P]
    pkv[0, 12] = b4[0]
    return pkv


def kernel(Xs, api_embeds, W1, b1, W2, b2, W3, b3, W4, b4, invoked):
    global LAST_RESULTS
    from concourse.bass_utils import run_bass_kernel_spmd

    if "nc" not in _NC_CACHE:
        _NC_CACHE["nc"] = build_nc()
    nc = _NC_CACHE["nc"]

    Xs = np.asarray(Xs, dtype=np.float32)
    emb = np.ascontiguousarray(np.asarray(api_embeds, dtype=np.float32))
    W1 = np.asarray(W1, dtype=np.float32)
    W2 = np.asarray(W2, dtype=np.float32)
    W3 = np.asarray(W3, dtype=np.float32)
    W4 = np.asarray(W4, dtype=np.float32).reshape(2 * D, 1)
    b1 = np.asarray(b1, dtype=np.float32).reshape(D)
    b2 = np.asarray(b2, dtype=np.float32).reshape(D)
    b3 = np.asarray(b3, dtype=np.float32).reshape(D)
    b4 = np.asarray(b4, dtype=np.float32).reshape(1)
    invoked = np.asarray(invoked, dtype=np.int32)

    pkw = _pack_w(W1, W2, W3, b2)
    in_maps = []
    for b in range(B):
        in_maps.append(
            {
                "invoked": np.ascontiguousarray(invoked[b].reshape(N, 1)),
                "emb": emb,
                "pkw": pkw,
                "pks": _pack_s(W4, b1, b2, b3, b4, Xs[b]),
            }
        )

    res = run_bass_kernel_spmd(nc, in_maps, core_ids=list(range(B)), trace=TRACE)
    LAST_RESULTS = res
    return np.stack([res.results[i]["out"] for i in range(B)], axis=0)


# revision 15
# speedup vs baseline: 1.0473x; 1.0473x over previous
"""Trainium2 Bass kernel for nn_AdaptiveHyperNN (gnn_message_passing).

Math: the reference builds fully-connected edge features [B,N,N,D] through
linear layers and mean-aggregates.  Every edge MLP is linear before
aggregation, so the computation collapses to per-node work:

  feat   = api_embeds[invoked]                       (indirect-DMA gather)
  e2n1_v = fbar @ W1a + feat_v @ W1b + b1            (fbar = mean_v feat_v)
  h_v    = feat_v @ W2a + e2n1_v @ W2b + b2
  logit[u*N+v] = p[u] + q[v] + c,  p = h @ (W3a@W4a), q = h @ (W3b@W4a),
                 c = b3 @ W4a + Xs @ W4b + b4
  out = sigmoid(q broadcast + p as per-partition ACT bias)

Sharding: data-parallel over B (8 graphs -> 8 cores), weights replicated.
Raw bacc with hand-placed semaphores:
- weights host-packed bf16, loaded by 3 large DMAs over the two HWDGE
  rings (sync + scalar) while the int32 indices load first and the
  gpsimd indirect gather runs
- all matmuls bf16 with f32 PSUM; fbar via an extra ones-column matmul;
  b2 injected into the h-PSUM by a rank-1 matmul; p fused into the
  sigmoid as a per-partition ACT bias; sigmoid table warmed up early
- sigmoid + output DMA split in half so the second half overlaps the
  first half's HBM receipt
- the bass-emitted start/end all-engine EVSEM barriers are stripped
  post-build (all ordering flows through this kernel's own semaphores)
"""

import numpy as np
import ml_dtypes

import concourse.bacc as bacc
import concourse.bass as bass
import concourse.mybir as mybir

P = 128
D = 256
N = 128
B = 8
V = 10000
F32 = mybir.dt.float32
BF16 = mybir.dt.bfloat16
I32 = mybir.dt.int32

_W1 = 0
_W2 = 1024
_W3T = 2048
PKW = 3328  # 3072 weights + 256 b2-row (partition 0)
PKS = 13


def build_nc():
    nc = bacc.Bacc("TRN2", target_bir_lowering=False)
    AO = mybir.AluOpType

    inv = nc.dram_tensor("invoked", [N, 1], I32, kind="ExternalInput")
    emb = nc.dram_tensor("emb", [V, D], F32, kind="ExternalInput")
    pkw = nc.dram_tensor("pkw", [P, PKW], BF16, kind="ExternalInput")
    pks = nc.dram_tensor("pks", [P, PKS], F32, kind="ExternalInput")
    out = nc.dram_tensor("out", [N * N, 1], F32, kind="ExternalOutput")

    sb = nc.alloc_sbuf_tensor
    ident = sb("ident", [P, P], F32)
    inv_t = sb("inv_t", [P, 1], I32)
    feat = sb("feat", [P, D], F32)
    sm_sb = sb("sm_sb", [P, PKS], F32)
    wa_sb = sb("wa_sb", [P, 1024], BF16)
    wb_sb = sb("wb_sb", [P, 1024], BF16)
    wc_sb = sb("wc_sb", [P, 1280], BF16)
    w4bf = sb("w4bf", [P, 2], BF16)
    ones_row = sb("ones_row", [1, P], BF16)
    ones_col = sb("ones_col", [P, 1], F32)
    featT = [sb(f"featT{i}", [P, P], BF16) for i in range(2)]
    fbar32 = [sb(f"fbar32_{i}", [P, 1], F32) for i in range(2)]
    fbar = [sb(f"fbar{i}", [P, 1], BF16) for i in range(2)]
    e_bias = [sb(f"ebias{i}", [P, 1], F32) for i in range(2)]
    e2n1T = [sb(f"e2n1T{i}", [P, P], BF16) for i in range(2)]
    hT = [sb(f"hT{i}", [P, P], BF16) for i in range(2)]
    w34c = [sb(f"w34c{i}", [P, 2], BF16) for i in range(2)]
    c_sb = sb("c_sb", [1, 1], F32)
    p_sb = sb("p_sb", [P, 1], F32)
    q_row = sb("q_row", [1, P], BF16)
    osb = sb("osb", [P, P], F32)
    warm2 = sb("warm2", [1, 1], F32)

    pp = nc.alloc_psum_tensor
    PB0 = pp("PB0", [P, P], F32)
    PB1 = pp("PB1", [P, P], F32)
    PB2 = pp("PB2", [P, P], F32)
    PB3 = pp("PB3", [P, P], F32)
    PS0 = pp("PS0", [P, 1], F32)
    PS1 = pp("PS1", [P, 1], F32)
    PCP = pp("PCP", [P, 1], F32)
    PQ = pp("PQ", [1, P], F32)

    def w1s(kt, mt):
        return wa_sb[:, kt * 256 + mt * 128 : kt * 256 + (mt + 1) * 128]

    def w2s(kt, mt):
        return wb_sb[:, kt * 256 + mt * 128 : kt * 256 + (mt + 1) * 128]

    def w3s(jt, it):
        return wc_sb[:, jt * 512 + it * 128 : jt * 512 + (it + 1) * 128]

    def w4s(kt):
        return sm_sb[:, kt : kt + 1]

    def b1s(k):
        return sm_sb[:, 4 + k : 5 + k]

    def b2s(k):
        return sm_sb[:, 6 + k : 7 + k]

    def b3s(k):
        return sm_sb[:, 8 + k : 9 + k]

    def xss(k):
        return sm_sb[:, 10 + k : 11 + k]

    TSF = mybir.ActivationFunctionType

    with (
        nc.Block() as block,
        nc.semaphore("dI") as dI,
        nc.semaphore("dWA") as dWA,
        nc.semaphore("dWC") as dWC,
        nc.semaphore("dSM") as dSM,
        nc.semaphore("dWB") as dWB,
        nc.semaphore("dOUT") as dOUT,
        nc.semaphore("dG") as dG,
        nc.semaphore("sG") as sG,
        nc.semaphore("sP") as sP,
        nc.semaphore("sV") as sV,
        nc.semaphore("sA") as sA,
    ):

        @block.sync
        def _(sync):
            sync.dma_start(out=inv_t[:], in_=inv[:, :], single_packet=True).then_inc(dI, 16)
            sync.dma_start(out=wc_sb[:], in_=pkw[:, _W3T : _W3T + 1280]).then_inc(dWC, 16)
            sync.dma_start(out=wa_sb[:], in_=pkw[:, _W1 : _W1 + 1024]).then_inc(dWA, 16)
            sync.wait_ge(sA, 1)
            sync.dma_start(
                out=out[0 : 64 * N, :].rearrange("(u v) o -> u (v o)", v=N),
                in_=osb[0:64, :],
            ).then_inc(dOUT, 16)
            sync.wait_ge(sA, 2)
            sync.dma_start(
                out=out[64 * N : 128 * N, :].rearrange("(u v) o -> u (v o)", v=N),
                in_=osb[64:128, :],
            ).then_inc(dOUT, 16)
            sync.wait_ge(dOUT, 32)

        @block.scalar
        def _(scalar):
            scalar.dma_start(out=sm_sb[:], in_=pks[:, :]).then_inc(dSM, 16)
            scalar.dma_start(out=wb_sb[:], in_=pkw[:, _W2 : _W2 + 1024]).then_inc(dWB, 16)
            scalar.wait_ge(dSM, 16)
            nc.scalar.activation(out=warm2[:], in_=sm_sb[0:1, 0:1], func=TSF.Sigmoid, bias=sm_sb[0:1, 1:2])
            scalar.wait_ge(sP, 18)
            scalar.wait_ge(sV, 18)
            nc.scalar.activation(
                out=osb[0:64, :], in_=PB2[0:64, :], func=TSF.Sigmoid,
                bias=p_sb[0:64, :1],
            ).then_inc(sA, 1)
            scalar.wait_ge(sP, 19)
            nc.scalar.activation(
                out=osb[64:128, :], in_=PB2[64:128, :], func=TSF.Sigmoid,
                bias=p_sb[64:128, :1],
            ).then_inc(sA, 1)

        @block.gpsimd
        def _(gpsimd):
            gpsimd.memset(ident[:], 0.0)
            gpsimd.drain()
            gpsimd.affine_select(
                out=ident[:],
                in_=ident[:],
                compare_op=mybir.AluOpType.not_equal,
                fill=1.0,
                base=0,
                pattern=[[-1, P]],
                channel_multiplier=1,
            ).then_inc(sG, 1)
            gpsimd.wait_ge(dI, 16)
            gpsimd.indirect_dma_start(
                out=feat[:],
                out_offset=None,
                in_=emb[:, :],
                in_offset=bass.IndirectOffsetOnAxis(ap=inv_t[:, :1], axis=0),
            ).then_inc(dG, 16)

        @block.tensor
        def _(tensor):
            mm = nc.tensor.matmul
            # w34 + c first: they only need weights, run during the gather wait
            tensor.wait_ge(dWC, 16)
            tensor.wait_ge(sV, 2)
            for it, ps_ in ((0, PS0), (1, PS1)):
                mm(out=ps_[:], lhsT=w3s(0, it), rhs=w4bf[:, 0:1], start=True, stop=False)
                mm(out=ps_[:], lhsT=w3s(1, it), rhs=w4bf[:, 1:2], start=False, stop=True).then_inc(sP, 1)
            tensor.wait_ge(sV, 3)
            mm(out=PS0[:], lhsT=w3s(0, 2), rhs=w4bf[:, 0:1], start=True, stop=False)
            mm(out=PS0[:], lhsT=w3s(1, 2), rhs=w4bf[:, 1:2], start=False, stop=True).then_inc(sP, 1)
            tensor.wait_ge(sV, 4)
            mm(out=PS1[:], lhsT=w3s(0, 3), rhs=w4bf[:, 0:1], start=True, stop=False)
            mm(out=PS1[:], lhsT=w3s(1, 3), rhs=w4bf[:, 1:2], start=False, stop=True).then_inc(sP, 1)
            mm(out=PCP[0:1, :], lhsT=b3s(0), rhs=w4s(0), start=True, stop=False)
            mm(out=PCP[0:1, :], lhsT=b3s(1), rhs=w4s(1), start=False, stop=False)
            mm(out=PCP[0:1, :], lhsT=xss(0), rhs=w4s(2), start=False, stop=False)
            mm(out=PCP[0:1, :], lhsT=xss(1), rhs=w4s(3), start=False, stop=True).then_inc(sP, 1)
            # gather-dependent chain; trXb computes fbar sums via ones column
            tensor.wait_ge(sG, 1)
            tensor.wait_ge(dG, 16)
            nc.tensor.transpose(out=PB0[:], in_=feat[:, 0:P], identity=ident[:]).then_inc(sP, 1)
            tensor.wait_ge(sV, 5)
            mm(out=PS0[:], lhsT=feat[:, 0:P], rhs=ones_col[:], start=True, stop=True).then_inc(sP, 1)
            nc.tensor.transpose(out=PB1[:], in_=feat[:, P : 2 * P], identity=ident[:]).then_inc(sP, 1)
            tensor.wait_ge(sV, 6)
            mm(out=PS1[:], lhsT=feat[:, P : 2 * P], rhs=ones_col[:], start=True, stop=True).then_inc(sP, 1)
            tensor.wait_ge(dWA, 16)
            tensor.wait_ge(sV, 8)
            mm(out=PB2[:], lhsT=w1s(2, 0), rhs=featT[0][:], start=True, stop=False)
            mm(out=PB3[:], lhsT=w1s(2, 1), rhs=featT[0][:], start=True, stop=False)
            tensor.wait_ge(sV, 10)
            mm(out=PB2[:], lhsT=w1s(3, 0), rhs=featT[1][:], start=False, stop=True).then_inc(sP, 1)
            mm(out=PB3[:], lhsT=w1s(3, 1), rhs=featT[1][:], start=False, stop=True).then_inc(sP, 1)
            tensor.wait_ge(sV, 11)
            mm(out=PS0[:], lhsT=w1s(0, 0), rhs=fbar[0][:], start=True, stop=False)
            mm(out=PS0[:], lhsT=w1s(1, 0), rhs=fbar[1][:], start=False, stop=True).then_inc(sP, 1)
            mm(out=PS1[:], lhsT=w1s(0, 1), rhs=fbar[0][:], start=True, stop=False)
            mm(out=PS1[:], lhsT=w1s(1, 1), rhs=fbar[1][:], start=False, stop=True).then_inc(sP, 1)
            tensor.wait_ge(dWB, 16)
            mm(out=PB0[:], lhsT=wc_sb[0:1, 1024:1152], rhs=ones_row[:], start=True, stop=False)
            mm(out=PB1[:], lhsT=wc_sb[0:1, 1152:1280], rhs=ones_row[:], start=True, stop=False)
            mm(out=PB0[:], lhsT=w2s(0, 0), rhs=featT[0][:], start=False, stop=False)
            mm(out=PB0[:], lhsT=w2s(1, 0), rhs=featT[1][:], start=False, stop=False)
            mm(out=PB1[:], lhsT=w2s(0, 1), rhs=featT[0][:], start=False, stop=False)
            mm(out=PB1[:], lhsT=w2s(1, 1), rhs=featT[1][:], start=False, stop=False)
            tensor.wait_ge(sV, 14)
            mm(out=PB0[:], lhsT=w2s(2, 0), rhs=e2n1T[0][:], start=False, stop=False)
            mm(out=PB1[:], lhsT=w2s(2, 1), rhs=e2n1T[0][:], start=False, stop=False)
            tensor.wait_ge(sV, 15)
            mm(out=PB0[:], lhsT=w2s(3, 0), rhs=e2n1T[1][:], start=False, stop=True).then_inc(sP, 1)
            mm(out=PB1[:], lhsT=w2s(3, 1), rhs=e2n1T[1][:], start=False, stop=True).then_inc(sP, 1)
            tensor.wait_ge(sV, 16)
            mm(out=PCP[:], lhsT=hT[0][:], rhs=w34c[0][:, 0:1], start=True, stop=False)
            tensor.wait_ge(sV, 17)
            mm(out=PCP[:], lhsT=hT[1][:], rhs=w34c[1][:, 0:1], start=False, stop=True).then_inc(sP, 1)
            mm(out=PQ[:], lhsT=w34c[0][:, 1:2], rhs=hT[0][:], start=True, stop=False)
            mm(out=PQ[:], lhsT=w34c[1][:, 1:2], rhs=hT[1][:], start=False, stop=True).then_inc(sP, 1)
            tensor.wait_ge(sV, 19)
            mm(out=PB2[0:64, :], lhsT=ones_row[:, 0:64], rhs=q_row[:], start=True, stop=True).then_inc(sP, 1)
            mm(out=PB2[64:128, :], lhsT=ones_row[:, 64:128], rhs=q_row[:], start=True, stop=True).then_inc(sP, 1)

        @block.vector
        def _(vector):
            nc.vector.memset(ones_row[:], 1.0).then_inc(sV, 1)
            vector.wait_ge(dSM, 16)
            nc.vector.tensor_copy(out=w4bf[:], in_=sm_sb[:, 0:2]).then_inc(sV, 1)
            nc.vector.memset(ones_col[:], 1.0)
            vector.wait_ge(sP, 1)
            nc.vector.tensor_copy(out=w34c[0][:, 0:1], in_=PS0[:]).then_inc(sV, 1)
            vector.wait_ge(sP, 2)
            nc.vector.tensor_copy(out=w34c[1][:, 0:1], in_=PS1[:]).then_inc(sV, 1)
            vector.wait_ge(sP, 3)
            nc.vector.tensor_copy(out=w34c[0][:, 1:2], in_=PS0[:]).then_inc(sV, 1)
            vector.wait_ge(sP, 4)
            nc.vector.tensor_copy(out=w34c[1][:, 1:2], in_=PS1[:]).then_inc(sV, 1)
            vector.wait_ge(sP, 5)
            nc.vector.tensor_add(out=c_sb[:], in0=PCP[0:1, :], in1=sm_sb[0:1, 12:13]).then_inc(sV, 1)
            vector.wait_ge(sP, 6)
            nc.vector.tensor_copy(out=featT[0][:], in_=PB0[:]).then_inc(sV, 1)
            vector.wait_ge(sP, 7)
            nc.vector.tensor_copy(out=fbar[0][:], in_=PS0[:]).then_inc(sV, 1)
            vector.wait_ge(sP, 8)
            nc.vector.tensor_copy(out=featT[1][:], in_=PB1[:]).then_inc(sV, 1)
            vector.wait_ge(sP, 9)
            nc.vector.tensor_copy(out=fbar[1][:], in_=PS1[:]).then_inc(sV, 1)
            vector.wait_ge(sP, 12)
            nc.vector.tensor_scalar(
                out=e_bias[0][:], in0=PS0[:], scalar1=1.0 / N, scalar2=b1s(0),
                op0=AO.mult, op1=AO.add,
            ).then_inc(sV, 1)
            vector.wait_ge(sP, 13)
            nc.vector.tensor_scalar(
                out=e_bias[1][:], in0=PS1[:], scalar1=1.0 / N, scalar2=b1s(1),
                op0=AO.mult, op1=AO.add,
            ).then_inc(sV, 1)
            vector.wait_ge(sP, 10)
            vector.wait_ge(sV, 12)
            nc.vector.tensor_scalar_add(out=e2n1T[0][:], in0=PB2[:], scalar1=e_bias[0][:, :1]).then_inc(sV, 1)
            vector.wait_ge(sP, 11)
            vector.wait_ge(sV, 13)
            nc.vector.tensor_scalar_add(out=e2n1T[1][:], in0=PB3[:], scalar1=e_bias[1][:, :1]).then_inc(sV, 1)
            vector.wait_ge(sP, 14)
            nc.vector.tensor_copy(out=hT[0][:], in_=PB0[:]).then_inc(sV, 1)
            vector.wait_ge(sP, 15)
            nc.vector.tensor_copy(out=hT[1][:], in_=PB1[:]).then_inc(sV, 1)
            vector.wait_ge(sP, 16)
            nc.vector.tensor_copy(out=p_sb[:], in_=PCP[:]).then_inc(sV, 1)
            vector.wait_ge(sP, 17)
            vector.wait_ge(sV, 7)
            nc.vector.tensor_scalar_add(out=q_row[:], in0=PQ[:], scalar1=c_sb[:1, :1]).then_inc(sV, 1)

    import concourse.mybir as _mb
    for bb in nc.m.functions[0].blocks:
        if bb.name == "main":
            bb.instructions = [
                i for i in bb.instructions
                if not i.name.startswith("barrier_")
                and not isinstance(i, _mb.InstDrain)
            ]
        elif bb.name.endswith("_end"):
            bb.instructions = [
                i for i in bb.instructions if not i.name.startswith("barrier_")
            ]
    nc.compile()
    return nc


TRACE = False
LAST_RESULTS = None
_NC_CACHE = {}


def _pack_w(W1, W2, W3, b2):
    pkv = np.zeros((P, PKW), dtype=ml_dtypes.bfloat16)
    for kt in range(4):
        pkv[:, _W1 + kt * 256 : _W1 + (kt + 1) * 256] = W1[kt * P : (kt + 1) * P, :]
        pkv[:, _W2 + kt * 256 : _W2 + (kt + 1) * 256] = W2[kt * P : (kt + 1) * P, :]
    W3T = W3.T
    for jt in range(2):
        pkv[:, _W3T + jt * 512 : _W3T + (jt + 1) * 512] = W3T[jt * P : (jt + 1) * P, :]
    pkv[0, 3072:3328] = b2
    return pkv


def _pack_s(W4, b1, b2, b3, b4, Xs_b):
    pkv = np.zeros((P, PKS), dtype=np.float32)
    for kt in range(4):
        pkv[:, kt] = W4[kt * P : (kt + 1) * P, 0]
    for jt in range(2):
        pkv[:, 4 + jt] = b1[jt * P : (jt + 1) * P]
        pkv[:, 6 + jt] = b2[jt * P : (jt + 1) * P]
        pkv[:, 8 + jt] = b3[jt * P : (jt + 1) * P]
        pkv[:, 10 + jt] = Xs_b[jt * P : (jt + 1) * P]
    pkv[0, 12] = b4[0]
    return pkv


def kernel(Xs, api_embeds, W1, b1, W2, b2, W3, b3, W4, b4, invoked):
    global LAST_RESULTS
    from concourse.bass_utils import run_bass_kernel_spmd

    if "nc" not in _NC_CACHE:
        _NC_CACHE["nc"] = build_nc()
    nc = _NC_CACHE["nc"]

    Xs = np.asarray(Xs, dtype=np.float32)
    emb = np.ascontiguousarray(np.asarray(api_embeds, dtype=np.float32))
    W1 = np.asarray(W1, dtype=np.float32)
    W2 = np.asarray(W2, dtype=np.float32)
    W3 = np.asarray(W3, dtype=np.float32)
    W4 = np.asarray(W4, dtype=np.float32).reshape(2 * D, 1)
    b1 = np.asarray(b1, dtype=np.float32).reshape(D)
    b2 = np.asarray(b2, dtype=np.float32).reshape(D)
    b3 = np.asarray(b3, dtype=np.float32).reshape(D)
    b4 = np.asarray(b4, dtype=np.float32).reshape(1)
    invoked = np.asarray(invoked, dtype=np.int32)

    pkw = _pack_w(W1, W2, W3, b2)
    in_maps = []
    for b in range(B):
        in_maps.append(
            {
                "invoked": np.ascontiguousarray(invoked[b].reshape(N, 1)),
                "emb": emb,
                "pkw": pkw,
                "pks": _pack_s(W4, b1, b2, b3, b4, Xs[b]),
            }
        )

    res = run_bass_kernel_spmd(nc, in_maps, core_ids=list(range(B)), trace=TRACE)
    LAST_RESULTS = res
    return np.stack([res.results[i]["out"] for i in range(B)], axis=0)
